# revision 25
# baseline (speedup 1.0000x reference)
"""MultiHeadAttention TRN2 Bass kernel.

Problem: B=16, L=1024, F=512, H=8 heads, D=64.
  q = Q@Wq+bq; k = K@Wk+bk; v = V@Wv+bv   (per-head split)
  S = q k^T / sqrt(D); P = softmax(S, axis=k); ctx = P v
  out = tanh(concat([ctx, Q]) @ Wo + bo)

Sharding: data-parallel over batch, 2 batches per core, 8 cores. No
collectives needed; full inputs sharded host-side, outputs gathered.

Device math (per core, fp16 compute / fp32 accumulate):
  - bk dropped entirely: its score contribution is constant along the
    softmax axis. bv folded into bo_eff = bo + bv @ Wo[:F] host-side.
  - QT/KT/VT loaded via XBAR DMA-transpose (fp16).
  - qT = Wq^T-proj(QT) + bq (bias via K=1 ones-row matmul);
    kT = Wk^T-proj(KT); v natural via VT-stationary matmul, stored with
    an appended ones column (v_aug) so the context matmul also yields
    the softmax denominator. Even heads: [v|1] -> ctx rows 0:64, denom
    row 64. Odd heads: [1|v] with psum base 63 -> denom row 63, ctx
    rows 64:128. This packs head pairs into full 128-partition ctxT
    chunks with no cross-partition copies.
  - scores computed transposed: S^T[k, q] = kT_h^T @ qT_h, exp on ACT
    (scale=1/8) straight out of PSUM into fp16 SBUF.
  - normalization: recip(denom) on DVE (fp16, values <= 1), partition-
    broadcast via a K=1 ones matmul, one DVE multiply per head.
  - out[qtile, :] = tanh( ctxT^T Wo_top + Q Wo_bot + bo_eff ), ctx
    matmuls take whole head pairs (K=128).
"""

import numpy as np

import concourse.bass as bass
import concourse.tile as tile
from concourse import bacc, mybir
from concourse import bass_utils

B, L, F, H, D = 16, 1024, 512, 8, 64
NCORES = 8
BPC = B // NCORES  # batches per core
NFC = F // 128     # feature chunks (4)
NST = L // 128     # seq tiles (8)
F16 = mybir.dt.float16
F32 = mybir.dt.float32

MM_N = 512        # moving free dim per matmul (PSUM bank limit)
PSA_BUFS = 2
PSB_BUFS = 2
EXP_BUFS = 2
KVT_BUFS = 2
PSR_POOL = "B"
PROJ_POOL = "B"
OUTP_POOL = "A"

_CACHE = {}


def _build_program():
    nc = bacc.Bacc("TRN2", target_bir_lowering=False)

    dQ = nc.dram_tensor("q_in", (BPC, L, F), F16, kind="ExternalInput")
    dK = nc.dram_tensor("k_in", (BPC, L, F), F16, kind="ExternalInput")
    dV = nc.dram_tensor("v_in", (BPC, L, F), F16, kind="ExternalInput")
    dWq = nc.dram_tensor("wq", (F, F), F16, kind="ExternalInput")
    dWk = nc.dram_tensor("wk", (F, F), F16, kind="ExternalInput")
    dWv = nc.dram_tensor("wv", (F, F), F16, kind="ExternalInput")
    dWoT = nc.dram_tensor("wo_top", (F, F), F16, kind="ExternalInput")
    dWoB = nc.dram_tensor("wo_bot", (F, F), F16, kind="ExternalInput")
    dbq = nc.dram_tensor("bq_row", (1, F), F16, kind="ExternalInput")
    dbo = nc.dram_tensor("bo_eff", (1, F), F16, kind="ExternalInput")
    dOut = nc.dram_tensor("out", (BPC, L, F), F32, kind="ExternalOutput")

    with tile.TileContext(nc) as tc:
        _kernel(tc, dQ, dK, dV, dWq, dWk, dWv, dWoT, dWoB, dbq, dbo, dOut)

    nc.compile()
    return nc


def _kernel(tc, dQ, dK, dV, dWq, dWk, dWv, dWoT, dWoB, dbq, dbo, dOut):
    nc = tc.nc
    Exp = mybir.ActivationFunctionType.Exp
    Tanh = mybir.ActivationFunctionType.Tanh

    from contextlib import ExitStack
    ctx = ExitStack()
    consts = ctx.enter_context(tc.tile_pool(name="consts", bufs=1))
    p_qt = ctx.enter_context(tc.tile_pool(name="p_qt", bufs=2))
    p_kvt = ctx.enter_context(tc.tile_pool(name="p_kvt", bufs=KVT_BUFS))
    p_proj = ctx.enter_context(tc.tile_pool(name="p_proj", bufs=2))
    p_exp = ctx.enter_context(tc.tile_pool(name="p_exp", bufs=EXP_BUFS))
    p_ctx = ctx.enter_context(tc.tile_pool(name="p_ctx", bufs=2))
    p_misc = ctx.enter_context(tc.tile_pool(name="p_misc", bufs=2))
    p_out = ctx.enter_context(tc.tile_pool(name="p_out", bufs=3))
    psA = ctx.enter_context(tc.tile_pool(name="psA", bufs=PSA_BUFS, space="PSUM"))
    psB = ctx.enter_context(tc.tile_pool(name="psB", bufs=PSB_BUFS, space="PSUM"))

    # ---- per-batch input transposes first: the first projection waits
    # on these, so issue them before the bulk of the weights.
    QTs, KTs, VTs = [], [], []
    for b in range(BPC):
        QT = p_qt.tile([128, NFC, L], F16, tag="QT")
        KT = p_kvt.tile([128, NFC, L], F16, tag="KT")
        VT = p_kvt.tile([128, NFC, L], F16, tag="VT")
        QTs.append(QT); KTs.append(KT); VTs.append(VT)

    for c in range(NFC):
        nc.sync.dma_start(out=QTs[0][:, c, :], in_=dQ[0, :, c * 128:(c + 1) * 128],
                          transpose=True)
    Wq_sb = consts.tile([128, NFC, F], F16, tag="wq")
    Wk_sb = consts.tile([128, NFC, F], F16, tag="wk")
    Wv_sb = consts.tile([128, NFC, F], F16, tag="wv")
    for c in range(NFC):
        nc.sync.dma_start(out=Wq_sb[:, c, :], in_=dWq[c * 128:(c + 1) * 128, :])
    bq_sb = consts.tile([1, F], F16, tag="bq")
    nc.sync.dma_start(out=bq_sb, in_=dbq[0:1, :])
    ones_sb = consts.tile([128, 1024], F16, tag="ones")
    nc.vector.memset(ones_sb, 1.0)
    for c in range(NFC):
        nc.sync.dma_start(out=KTs[0][:, c, :], in_=dK[0, :, c * 128:(c + 1) * 128],
                         transpose=True)
        nc.sync.dma_start(out=VTs[0][:, c, :], in_=dV[0, :, c * 128:(c + 1) * 128],
                         transpose=True)
    for c in range(NFC):
        nc.sync.dma_start(out=Wk_sb[:, c, :], in_=dWk[c * 128:(c + 1) * 128, :])
        nc.sync.dma_start(out=Wv_sb[:, c, :], in_=dWv[c * 128:(c + 1) * 128, :])
    # Wo top half per-head at partition base 0; bottom in 128-row chunks
    WoT_sb = consts.tile([128, H, F], F16, tag="wot")
    for h in range(H):
        nc.sync.dma_start(out=WoT_sb[0:D, h, :], in_=dWoT[h * D:(h + 1) * D, :])
    WoB_sb = consts.tile([128, NFC, F], F16, tag="wob")
    for c in range(NFC):
        nc.sync.dma_start(out=WoB_sb[:, c, :], in_=dWoB[c * 128:(c + 1) * 128, :])
    bo_sb = consts.tile([1, F], F16, tag="bo")
    nc.sync.dma_start(out=bo_sb, in_=dbo[0:1, :])
    for b in range(1, BPC):
        for c in range(NFC):
            nc.sync.dma_start(out=QTs[b][:, c, :],
                              in_=dQ[b, :, c * 128:(c + 1) * 128], transpose=True)
            nc.sync.dma_start(out=KTs[b][:, c, :],
                             in_=dK[b, :, c * 128:(c + 1) * 128], transpose=True)
            nc.sync.dma_start(out=VTs[b][:, c, :],
                             in_=dV[b, :, c * 128:(c + 1) * 128], transpose=True)

    NH = L // MM_N  # moving-dim chunks per full seq (1 when MM_N=1024)

    for b in range(BPC):
        QT, KT, VT = QTs[b], KTs[b], VTs[b]

        # ---- projections ------------------------------------------
        qT = p_proj.tile([128, NFC, L], F16, tag="qT")
        kT = p_proj.tile([128, NFC, L], F16, tag="kT")
        # per head: [v(64) | one] -> ctx rows 0:64, denom row 64
        vaug = p_proj.tile([128, NST, H, D + 1], F16, tag="vaug")
        nc.vector.memset(vaug[:, :, :, D:D + 1], 1.0)

        for fo in range(NFC):
            ps = (psA if PROJ_POOL == "A" else psB).tile([128, 1024], F32, tag=PROJ_POOL)
            for nh in range(NH):
                o = ps[:, nh * MM_N:(nh + 1) * MM_N]
                for c in range(NFC):
                    nc.tensor.matmul(o, Wq_sb[:, c, fo * 128:(fo + 1) * 128],
                                     QT[:, c, nh * MM_N:(nh + 1) * MM_N],
                                     start=(c == 0), stop=False)
                nc.tensor.matmul(o, bq_sb[0:1, fo * 128:(fo + 1) * 128],
                                 ones_sb[0:1, 0:MM_N], start=False, stop=True)
            nc.vector.tensor_copy(out=qT[:, fo, :], in_=ps)

        for fo in range(NFC):
            ps = (psA if PROJ_POOL == "A" else psB).tile([128, 1024], F32, tag=PROJ_POOL)
            for nh in range(NH):
                o = ps[:, nh * MM_N:(nh + 1) * MM_N]
                for c in range(NFC):
                    nc.tensor.matmul(o, Wk_sb[:, c, fo * 128:(fo + 1) * 128],
                                     KT[:, c, nh * MM_N:(nh + 1) * MM_N],
                                     start=(c == 0), stop=(c == NFC - 1))
            nc.vector.tensor_copy(out=kT[:, fo, :], in_=ps)

        for st in range(NST):
            ps = (psA if PROJ_POOL == "A" else psB).tile([128, 1024], F32, tag=PROJ_POOL)
            for c in range(NFC):
                nc.tensor.matmul(ps[:, 0:512], VT[:, c, st * 128:(st + 1) * 128],
                                 Wv_sb[:, c, :], start=(c == 0), stop=(c == NFC - 1))
            nc.vector.tensor_copy(
                out=vaug[:, st, :, 0:D],
                in_=ps[:, 0:512].rearrange("p (h d) -> p h d", h=H))

        # ---- attention --------------------------------------------
        # ctxT: one head per chunk, partitions 0:64 (normalized, fp16)
        ctxT = p_ctx.tile([128, H, L], F16, tag="ctxT")
        for h in range(H):
            hb = (h % 2) * 64
            hc = h // 2
            hc2 = h
            expS = p_exp.tile([128, NST, L], F16, tag="expS")
            for kt in range(NST):
                ps = psA.tile([128, 1024], F32, tag="A")
                for nh in range(NH):
                    nc.tensor.matmul(
                        ps[:, nh * MM_N:(nh + 1) * MM_N],
                        kT[hb:hb + D, hc, kt * 128:(kt + 1) * 128],
                        qT[hb:hb + D, hc, nh * MM_N:(nh + 1) * MM_N],
                        start=True, stop=True)
                nc.scalar.activation(out=expS[:, kt, :], in_=ps, func=Exp,
                                     scale=0.125)

            psc = psB.tile([128, 1024], F32, tag="B")
            for nh in range(NH):
                o = psc[0:D + 1, nh * MM_N:(nh + 1) * MM_N]
                for kt in range(NST):
                    nc.tensor.matmul(o, vaug[:, kt, h, :],
                                     expS[:, kt, nh * MM_N:(nh + 1) * MM_N],
                                     start=(kt == 0), stop=(kt == NST - 1))
            recip = p_misc.tile([128, 1024], F16, tag="recip")
            with nc.allow_low_precision(reason="softmax recip <=1, fp16 ok"):
                nc.vector.reciprocal(out=recip[D:D + 1, :],
                                     in_=psc[D:D + 1, :])
            psr = (psA if PSR_POOL == "A" else psB).tile([128, 1024], F32, tag=PSR_POOL)
            for half in range(2):
                nc.tensor.matmul(psr[0:D, half * 512:(half + 1) * 512],
                                 ones_sb[D:D + 1, 0:D],
                                 recip[D:D + 1, half * 512:(half + 1) * 512],
                                 start=True, stop=True)
            nc.vector.tensor_copy(out=ctxT[0:D, hc2, :], in_=psc[0:D, :])
            nc.vector.tensor_mul(out=ctxT[0:D, hc2, :], in0=psr[0:D, :],
                                 in1=ctxT[0:D, hc2, :])

        # ---- output projection ------------------------------------
        for qt in range(NST):
            ps = (psA if OUTP_POOL == "A" else psB).tile([128, 1024], F32, tag=OUTP_POOL)
            o = ps[:, 0:512]
            for h in range(H):
                nc.tensor.matmul(o, ctxT[0:D, h, qt * 128:(qt + 1) * 128],
                                 WoT_sb[0:D, h, :], start=(h == 0), stop=False)
            for c in range(NFC):
                nc.tensor.matmul(o, QT[:, c, qt * 128:(qt + 1) * 128],
                                 WoB_sb[:, c, :], start=False, stop=False)
            nc.tensor.matmul(o, ones_sb[0:1, 0:128],
                             bo_sb[0:1, :], start=False, stop=True)
            out_t = p_out.tile([128, 512], F32, tag="out")
            nc.scalar.activation(out=out_t, in_=o, func=Tanh)
            nc.sync.dma_start(out=dOut[b, qt * 128:(qt + 1) * 128, :], in_=out_t)

    ctx.close()


def kernel(Q, K, V, Wq, bq, Wk, bk, Wv, bv, Wo, bo):
    if "nc" not in _CACHE:
        _CACHE["nc"] = _build_program()
    nc = _CACHE["nc"]

    Q, K, V = (np.asarray(x, dtype=np.float32) for x in (Q, K, V))
    Wq, Wk, Wv, Wo = (np.asarray(x, dtype=np.float32) for x in (Wq, Wk, Wv, Wo))
    bq, bv, bo = (np.asarray(x, dtype=np.float32) for x in (bq, bv, bo))

    f32 = np.float32
    Wo_top = Wo[:F, :].astype(f32)
    bo_eff = bo.astype(f32) + bv.astype(f32) @ Wo_top  # bv folds through Wo
    h16 = np.float16

    in_common = {
        "wq": Wq.astype(h16), "wk": Wk.astype(h16), "wv": Wv.astype(h16),
        "wo_top": Wo_top.astype(h16), "wo_bot": Wo[F:, :].astype(h16),
        "bq_row": bq.reshape(1, F).astype(h16),
        "bo_eff": bo_eff.reshape(1, F).astype(h16),
    }
    Qh = np.asarray(Q, dtype=h16)
    Kh = np.asarray(K, dtype=h16)
    Vh = np.asarray(V, dtype=h16)
    in_maps = []
    for c in range(NCORES):
        s = slice(c * BPC, (c + 1) * BPC)
        in_maps.append({"q_in": Qh[s], "k_in": Kh[s], "v_in": Vh[s], **in_common})

    _CACHE["in_maps"] = in_maps
    res = bass_utils.run_bass_kernel_spmd(nc, in_maps, core_ids=list(range(NCORES)))
    out = np.concatenate([r["out"] for r in res.results], axis=0)
    return out.astype(np.float32)


def _last_in_maps():
    return _CACHE["in_maps"]
